# revision 66
# baseline (speedup 1.0000x reference)
"""Causal self-attention (GPT-style, 12 heads, C=768) on 8 TRN2 NeuronCores.

Sharding: core c -> (batch b = c//2, head-group g = c%2 of 6 heads).
Each core computes qkv projection for its 6 heads, causal attention, and a
partial output projection (its 384 rows of w_proj). Host sums the two
partial projections per batch (row-parallel tensor parallelism); b_proj is
folded into the g=0 core's partial.

v3 additions over v2:
  - q/k projection runs in fp8(e4m3) DoubleRow perf mode: x and w_attn's
    q/k columns are shipped in fp8 alongside bf16; 3 double-row matmuls
    replace 6 bf16 k-tile matmuls (v projection stays bf16 for accuracy).
  - avt-psum evacuation (S row + outT casts) is deferred into the next
    pair's first-exp slot, so the next pair's exp is never queued behind
    the copies and fillers bridge the psum-release latency.
  - y stores ride the gpsimd DMA queue (one merged store per row-tile);
    scalar/sync queues never delay exp-critical work.
  - chunk-3 1/S is split: pairs 0-1 normalize inside pair 2's window,
    pair 2 normalizes in a keepalive-bridged tail.
  - warmup trimmed to 32 matmuls (input DMAs land ~2.5us after start).

v2 pipeline design (HAM-warm scheduling):
  - score psum groups are per-j-tile [128, 2(heads), 512] tiles (2 banks),
    ring of 2 -> depth-2 software pipeline: scores(g+1) issue while exp(g)
    runs on ScalarE, AV(g) follows. One exp op covers both heads.
  - AV accumulates into one [65, 2, 512] psum (M=65: V plus a ones column
    so row 64 accumulates the softmax denominator S).
  - softmax normalization is deferred: av rows 0..63 are copied to outT
    unnormalized; S rows are gathered (partitions 0/32/64 of an SBUF
    table) and 1/S = exp(-ln S) is computed once per chunk in one batched
    Ln + Exp pair. Per (pair, head): K=1 broadcast matmul of 1/S then one
    in-place DVE multiply on outT.
  - independent matmul work (next chunk's qkv projection, previous chunk's
    normalize broadcasts + output projection) is interleaved between
    attention groups as PE filler so the tensor engine never idles long
    enough for the HAM clock gate to re-throttle it to 1.2 GHz.
  - single shared 2-slot PSUM scratch ring serves qkv/proj/broadcast;
    8 PSUM banks total: 4 (scores) + 2 (AV) + 2 (scratch).
"""

import numpy as np

import concourse.bass as bass
import concourse.mybir as mybir
import concourse.tile as tile
from concourse import bacc
from concourse import bass_utils

f32 = mybir.dt.float32
bf16 = mybir.dt.bfloat16
f8 = mybir.dt.float8e4
AF = mybir.ActivationFunctionType
ALU = mybir.AluOpType
DR = mybir.MatmulPerfMode.DoubleRow

N_HEAD = 12
N_EMBD = 768
B_FULL = 4
T_FULL = 2048
N_CORES = 8
SCALE = float(N_EMBD) ** -0.5

TRACE = False
LAST_RESULT = None
_NC_CACHE = {}


def build_nc(T=T_FULL, dbg=False):
    C = N_EMBD            # 768
    NP = 3                # head pairs (6 local heads)
    KT = C // 128         # 6 k-tiles for the projections
    NIC = T // 512        # i-chunks (512 queries each)
    NJT = T // 128        # j-tiles (128 keys each)

    # Pin Exp/Ln to the one activation-table set containing both, so the
    # table-load pass emits a single load.
    import concourse.bacc as _bacc_mod
    from concourse.hw_specs import get_activation_tables as _orig_gat

    def _pinned_gat(arch):
        tabs = {k: set(v) for k, v in _orig_gat(arch).items()}
        for name, fns in tabs.items():
            if name != "natural_log_exp_and_others":
                fns.discard(AF.Exp)
                fns.discard(AF.Ln)
        return tabs

    nc = bacc.Bacc("TRN2", target_bir_lowering=False, debug=False)

    # inputs are pre-packed on the host into [128 partitions, ...] layouts
    # whose per-partition DMA lines are 2.3-6KB contiguous: small (<1KB)
    # lines run at ~21GB/s/queue on trn2, large ones several times faster
    xT_d = nc.dram_tensor("xT", [128, NIC * KT * 512], bf16,
                          kind="ExternalInput")
    xf8_d = nc.dram_tensor("xf8", [128, NIC * KT * 512], f8,
                           kind="ExternalInput")
    wqk_d = nc.dram_tensor("wqk", [128, KT * 768], f8, kind="ExternalInput")
    wv_d = nc.dram_tensor("wv", [128, KT * 384], bf16, kind="ExternalInput")
    wp_d = nc.dram_tensor("wp", [128, NP * 768], bf16, kind="ExternalInput")
    bqk_d = nc.dram_tensor("bqk", [128, 6], f32, kind="ExternalInput")
    # blob: all small bf16 constants in one transfer (each dma_start costs
    # ~2.4us fixed on its queue, so small consts are batched): cols
    # [0:128) ones, [128:384) causal mask (2 heads x 128), [384:768) bvb
    # (v-bias broadcast to 128 rows), [768:1536) bpb (proj-bias broadcast)
    blob_d = nc.dram_tensor("blob", [128, 1536], bf16, kind="ExternalInput")
    y_d = nc.dram_tensor("y", [T, C], bf16, kind="ExternalOutput")

    with tile.TileContext(nc) as tc:
        with (
            tc.tile_pool(name="const", bufs=1) as constp,
            tc.tile_pool(name="xt", bufs=1) as xtp,
            tc.tile_pool(name="qk", bufs=1) as qkp,
            tc.tile_pool(name="vs", bufs=16) as vsp,
            tc.tile_pool(name="es", bufs=10) as esp,
            tc.tile_pool(name="ot", bufs=1) as otp,
            tc.tile_pool(name="sn", bufs=1) as snp,
            tc.tile_pool(name="ys", bufs=4) as ysp,
            tc.tile_pool(name="psg", bufs=2, space="PSUM") as psgp,
            tc.tile_pool(name="pav", bufs=1, space="PSUM") as pavp,
            tc.tile_pool(name="psc", bufs=2, space="PSUM") as pscp,
        ):
            # ---------------- startup constants + prefetch ----------------
            # HAM warm-up: dummy matmuls on a memset tile (no DMA dep) trip
            # the PE clock gate to 8/8 before the first real matmuls arrive.
            warm_src = constp.tile([128, 128], bf16, tag="wsrc")
            nc.gpsimd.memset(warm_src[:], 0.5)
            warm = psgp.tile([128, 2, 512], f32, tag="sg", name="warm")
            for i in range(42):
                nc.tensor.matmul(warm[:, i % 2, 0:128], warm_src[:],
                                 warm_src[:], start=True, stop=True)

            # DMA plan: one big dma_start per tensor (fixed ~2.4us cost per
            # dma_start dominates), critical transfers first per queue:
            #   sync:   wqk(f8)  -> wv    -> xf8 chunks 1-3 (merged)
            #   scalar: xf8[0]   -> blob
            #   gpsimd: bqk      -> xt0   -> wp -> xt chunks 1-3 (merged)
            wqk_t = constp.tile([128, KT, 768], f8, tag="wqk")
            wqk_src = wqk_d.ap().rearrange("p (k c) -> p k c", k=KT)

            xts_all = xtp.tile([128, NIC, KT, 512], bf16, tag="xt")
            xf8_all = xtp.tile([128, NIC, KT, 512], f8, tag="xf")
            xts_tiles = [xts_all[:, ic] for ic in range(NIC)]
            xf8_tiles = [xf8_all[:, ic] for ic in range(NIC)]
            xsrc = xT_d.ap().rearrange("p (i k t) -> p i k t", i=NIC, k=KT)
            xf8src = xf8_d.ap().rearrange("p (i k t) -> p i k t",
                                          i=NIC, k=KT)

            # critical transfers (wqk + xf8[0]) split across all three
            # queues so the first qk unit's inputs land together ASAP;
            # xf8[1] rides early so chunk-1 qk fillers can cover the window
            # while the (bigger) bf16 xt0 + wv still stream in
            wv_t = constp.tile([128, KT, 384], bf16, tag="wv")
            wv_src = wv_d.ap().rearrange("p (k c) -> p k c", k=KT)
            blob = constp.tile([128, 1536], bf16, tag="blob")
            bqk_t = constp.tile([128, 6], f32, tag="bqk")

            nc.sync.dma_start(wqk_t[:, 0:4, :], wqk_src[:, 0:4, :])
            nc.gpsimd.dma_start(wqk_t[:, 4:6, :], wqk_src[:, 4:6, :])
            nc.scalar.dma_start(xf8_all[:, 0], xf8src[:, 0])
            nc.sync.dma_start(xf8_all[:, 1, 0:3], xf8src[:, 1, 0:3])
            nc.scalar.dma_start(xf8_all[:, 1, 3:6], xf8src[:, 1, 3:6])
            nc.gpsimd.dma_start(bqk_t[:], bqk_d.ap()[:])
            bqk = [bqk_t[:, m:m + 1] for m in range(6)]
            nc.sync.dma_start(xts_all[:, 0, 0:3], xsrc[:, 0, 0:3])
            nc.gpsimd.dma_start(xts_all[:, 0, 3:6], xsrc[:, 0, 3:6])
            nc.scalar.dma_start(blob[:], blob_d.ap()[:])
            nc.sync.dma_start(wv_t[:, 0:3, :], wv_src[:, 0:3, :])
            nc.gpsimd.dma_start(wv_t[:, 3:6, :], wv_src[:, 3:6, :])

            ones = blob[:, 0:128]
            bvb_r = blob[:, 384:768].rearrange("p (h d) -> p h d", h=6)
            wv = [wv_t[:, k, :] for k in range(KT)]

            wp_t = constp.tile([128, NP, 768], bf16, tag="wp")
            wp = [wp_t[:, m, :] for m in range(NP)]
            msk = blob[:, 128:384].rearrange("p (h c) -> p h c", h=2)
            bpb = blob[:, 768:1536]

            def gated_prefetch():
                # merged chunk 1-3 prefetches + proj weights, WAW-gated by
                # DVE memsets so the ~3MB doesn't contend on the shared
                # SDMA engines with the startup-critical transfers; xt[1]
                # goes alone first (v-unit(1) fillers need it soonest)
                nc.vector.memset(xts_all[0:1, 1:2, 0:1, 0:1], 0.0)
                nc.vector.memset(xf8_all[0:1, 2:3, 0:1, 0:1], 0.0)
                nc.gpsimd.dma_start(xts_all[:, 1], xsrc[:, 1])
                nc.sync.dma_start(xf8_all[:, 2:4], xf8src[:, 2:4])
                nc.gpsimd.dma_start(xts_all[:, 2:4], xsrc[:, 2:4])
                nc.gpsimd.dma_start(
                    wp_t[:], wp_d.ap().rearrange("p (m c) -> p m c", m=NP))

            # softmax-denominator table: rows at partitions 32p hold, per
            # chunk, [2 heads x 512] sums; memset so Ln of unused rows is
            # well-defined.
            S_all = snp.tile([65, NIC, 2, 512], f32, tag="sall")
            nc.gpsimd.memset(S_all[:], 1.0)
            rr_all = snp.tile([65, NIC, 2, 512], bf16, tag="rrall")

            # v tiles: [keys 128, local head, 64 v-dims + ones column]
            v = [vsp.tile([128, 6, 65], bf16, tag="v", name=f"v{j}")
                 for j in range(NJT)]
            for j in range(NJT):
                nc.vector.memset(v[j][:, :, 64:65], 1.0)

            # ---------------- qkv projection units ------------------------
            qT = [qkp.tile([128, T], bf16, tag=f"qT{p}", name=f"qT{p}")
                  for p in range(NP)]
            kT = [qkp.tile([128, T], bf16, tag=f"kT{p}", name=f"kT{p}")
                  for p in range(NP)]

            def qk_unit(tci, m):
                def emit():
                    ps = pscp.tile([128, 512], f32, tag="sc",
                                   name=f"psqk{tci}_{m}")
                    xf = xf8_tiles[tci]
                    msl = slice(128 * m, 128 * (m + 1))
                    for t in range(KT // 2):
                        nc.tensor.matmul(ps[:],
                                         wqk_t[:, 2 * t:2 * t + 2, msl],
                                         xf[:, 2 * t:2 * t + 2, :],
                                         start=(t == 0), stop=(t == KT // 2 - 1),
                                         perf_mode=DR)
                    dest = qT[m] if m < 3 else kT[m - 3]
                    nc.vector.tensor_scalar_add(
                        dest[:, 512 * tci:512 * (tci + 1)], ps[:], bqk[m])
                return emit

            def v_unit(tci, tsub):
                def emit():
                    ps = pscp.tile([128, 512], f32, tag="sc",
                                   name=f"psv{tci}_{tsub}")
                    xts = xts_tiles[tci]
                    jt = 4 * tci + tsub
                    for k in range(KT):
                        nc.tensor.matmul(
                            ps[:, 0:384],
                            xts[:, k, 128 * tsub:128 * (tsub + 1)],
                            wv[k], start=(k == 0), stop=(k == KT - 1))
                    nc.vector.tensor_tensor(
                        v[jt][:, :, 0:64],
                        ps[:, 0:384].rearrange("p (h d) -> p h d", h=6),
                        bvb_r, op=ALU.add)
                return emit

            def qkv_units(tci):
                us = [qk_unit(tci, m) for m in range(6)]
                us += [v_unit(tci, tsub) for tsub in range(4)]
                return us

            def q_units(tci):
                return [qk_unit(tci, m) for m in range(3)]

            def kv_units(tci):
                return ([qk_unit(tci, m) for m in range(3, 6)]
                        + [v_unit(tci, tsub) for tsub in range(4)])

            outT = [otp.tile([128, T], bf16, tag=f"outT{p}", name=f"outT{p}")
                    for p in range(NP)]

            # ---------------- normalize + projection units ----------------
            def norm_unit(ic, p, h):
                def emit():
                    dsl = slice(64 * h, 64 * (h + 1))
                    isl = slice(512 * ic, 512 * (ic + 1))
                    sl = pscp.tile([128, 512], f32, tag="sc",
                                   name=f"rbp{ic}_{p}_{h}")
                    rbp = sl[0:64, :]
                    nc.tensor.matmul(rbp,
                                     ones[32 * p:32 * p + 1, 0:64],
                                     rr_all[32 * p:32 * p + 1, ic, h, :],
                                     start=True, stop=True)
                    nc.vector.tensor_tensor(outT[p][dsl, isl],
                                            outT[p][dsl, isl], rbp,
                                            op=ALU.mult)
                return emit

            def proj_half(ic, tsub, n, st, pool=None):
                # half a projection unit (one 384-col n-slice): ~0.5us of PE
                # work per filler quantum; each half owns its own psc slot so
                # halves from different units can interleave safely
                def emit():
                    t0 = 512 * ic + 128 * tsub
                    if n == 0:
                        st["ysb"] = ysp.tile([128, 768], bf16, tag="y",
                                             name=f"y{ic}_{tsub}")
                    ysb = st["ysb"]
                    nsl = slice(384 * n, 384 * (n + 1))
                    pl = pool if pool is not None else pscp
                    sl = pl.tile([128, 512], f32,
                                 tag="sc" if pl is pscp else "sg",
                                 name=f"yp{ic}_{tsub}_{n}")
                    yp = sl[:, 0:384]
                    for mp in range(NP):
                        nc.tensor.matmul(
                            yp, outT[mp][:, t0:t0 + 128], wp[mp][:, nsl],
                            start=(mp == 0), stop=(mp == NP - 1))
                    nc.vector.tensor_tensor(ysb[:, nsl], yp,
                                            bpb[:, nsl], op=ALU.add)
                    if ic == NIC - 1:
                        # final chunk: store each half as soon as its bias
                        # add completes (HWDGE queues), so the end-of-kernel
                        # drain only waits on the last ~200KB
                        eng = nc.sync if n == 0 else nc.scalar
                        eng.dma_start(y_d.ap()[t0:t0 + 128, nsl],
                                      ysb[:, nsl])
                    elif n == 1:
                        eng = nc.gpsimd if tsub % 2 == 0 else nc.sync
                        eng.dma_start(y_d.ap()[t0:t0 + 128, :], ysb[:])
                return emit

            def proj_unit(ic, tsub, pool=None):
                st = {}
                halves = [proj_half(ic, tsub, n, st, pool) for n in range(2)]

                def emit():
                    for hf in halves:
                        hf()
                return emit

            def norm_proj_units(ic):
                us = [norm_unit(ic, p, h) for p in range(NP) for h in (0, 1)]
                for tsub in range(4):
                    st = {}
                    us += [proj_half(ic, tsub, n, st) for n in range(2)]
                return us

            # ---------------- attention pairs -----------------------------
            pairs = [(0, slice(0, 64)), (1, slice(64, 128))]

            class Filler:
                def __init__(self):
                    self.units = []
                    self.credit = 0.0
                    self.quota = 0.0

                def add(self, us, groups_left):
                    self.units.extend(us)
                    self.quota = len(self.units) / max(groups_left, 1)

                def step(self):
                    self.credit += self.quota
                    while self.units and self.credit >= 1.0:
                        self.units.pop(0)()
                        self.credit -= 1.0

                def flush(self):
                    for u in self.units:
                        u()
                    self.units = []

            def emit_scores(ic, p, g):
                isl = slice(512 * ic, 512 * (ic + 1))
                kind = g[0]
                if kind in ("full", "d0"):
                    jt = g[1] if kind == "full" else 4 * ic
                    sg = psgp.tile([128, 2, 512], f32, tag="sg",
                                   name=f"sg{ic}_{p}_{kind}{jt}")
                    for h, dsl in pairs:
                        nc.tensor.matmul(
                            sg[:, h, :],
                            kT[p][dsl, 128 * jt:128 * (jt + 1)],
                            qT[p][dsl, isl], start=True, stop=True)
                    return (sg, 512)
                if kind == "d1":
                    jt = 4 * ic + 1
                    sg = psgp.tile([128, 2, 512], f32, tag="sg",
                                   name=f"sgd1_{ic}_{p}")
                    for h, dsl in pairs:
                        nc.tensor.matmul(
                            sg[:, h, 0:384],
                            kT[p][dsl, 128 * jt:128 * (jt + 1)],
                            qT[p][dsl, 512 * ic + 128:512 * ic + 512],
                            start=True, stop=True)
                    return (sg, 384)
                # d23: two decreasing blocks packed per head
                sg = psgp.tile([128, 2, 512], f32, tag="sg",
                               name=f"sgd23_{ic}_{p}")
                for h, dsl in pairs:
                    nc.tensor.matmul(
                        sg[:, h, 0:256],
                        kT[p][dsl, 128 * (4 * ic + 2):128 * (4 * ic + 3)],
                        qT[p][dsl, 512 * ic + 256:512 * ic + 512],
                        start=True, stop=False)
                    nc.tensor.matmul(
                        sg[:, h, 256:384],
                        kT[p][dsl, 128 * (4 * ic + 3):128 * (4 * ic + 4)],
                        qT[p][dsl, 512 * ic + 384:512 * ic + 512],
                        start=False, stop=True)
                return (sg, 384)

            def emit_exp(ic, p, g, sg_w):
                kind = g[0]
                sg, w = sg_w
                et = esp.tile([128, 2, w], bf16, tag="et",
                              name=f"et{ic}_{p}_{kind}")
                nc.scalar.activation(et[:], sg[:, :, 0:w], AF.Exp,
                                     scale=SCALE)
                # causal-mask multiplies stay on DVE: they gate the AV
                # matmuls, and the gpsimd queue's ~0.7us DMA descriptors
                # would delay them at exactly the wrong moments
                if kind in ("d0", "d1", "d23"):
                    nc.vector.tensor_tensor(et[:, :, 0:128],
                                            et[:, :, 0:128], msk[:],
                                            op=ALU.mult)
                if kind == "d23":
                    nc.vector.tensor_tensor(et[:, :, 256:384],
                                            et[:, :, 256:384], msk[:],
                                            op=ALU.mult)
                return et

            def emit_av(ic, p, g, et, avt, first):
                kind = g[0]
                for h, dsl in pairs:
                    hl = 2 * p + h
                    if kind == "full":
                        nc.tensor.matmul(avt[:, h, :], v[g[1]][:, hl, :],
                                         et[:, h, :],
                                         start=first, stop=False)
                    elif kind == "d0":
                        nc.tensor.matmul(avt[:, h, :],
                                         v[4 * ic][:, hl, :], et[:, h, :],
                                         start=first, stop=False)
                    elif kind == "d1":
                        nc.tensor.matmul(avt[:, h, 128:512],
                                         v[4 * ic + 1][:, hl, :],
                                         et[:, h, :],
                                         start=False, stop=False)
                    else:
                        nc.tensor.matmul(avt[:, h, 256:512],
                                         v[4 * ic + 2][:, hl, :],
                                         et[:, h, 0:256],
                                         start=False, stop=False)
                        nc.tensor.matmul(avt[:, h, 384:512],
                                         v[4 * ic + 3][:, hl, :],
                                         et[:, h, 256:384],
                                         start=False, stop=True)

            def emit_lnexp(ic, rows=slice(0, 65)):
                if ic == 0:
                    # probe: DVE hardware reciprocal (off the ACT engine);
                    # chunk-0's rr has a whole chunk of slack before use
                    with nc.allow_low_precision(reason="1/S in bf16"):
                        nc.vector.reciprocal(rr_all[rows, ic],
                                             S_all[rows, ic])
                    return
                # rr = exp(-ln S): batched 1/S over the S rows of a chunk
                lnS = snp.tile([65, 2, 512], f32, tag="lnS", bufs=2,
                               name=f"lnS{ic}_{rows.start}")
                n = rows.stop - rows.start
                nc.scalar.activation(lnS[0:n], S_all[rows, ic], AF.Ln)
                nc.scalar.activation(rr_all[rows, ic], lnS[0:n],
                                     AF.Exp, scale=-1.0)

            def make_evac(ic, p, avt, extra=None):
                # avt-psum evacuation closure: S row first (it gates the 1/S
                # chain in `extra`), then the unnormalized outT casts.
                isl = slice(512 * ic, 512 * (ic + 1))

                def emit():
                    nc.vector.tensor_copy(S_all[32 * p:32 * p + 1, ic],
                                          avt[64:65, :, :])
                    for h, dsl in pairs:
                        nc.vector.tensor_copy(outT[p][dsl, isl],
                                              avt[0:64, h, :])
                    if extra is not None:
                        extra()
                return emit

            def first_group(ic):
                return ("d0",) if ic == 0 else ("full", 0)

            def emit_attn_pair(ic, p, fill, prev_evac=None,
                               first_sg=None, lookahead=None):
                # returns the next pair's pre-issued first score group (or
                # None) and this pair's avt psum tile (for deferred evac)
                avt = pavp.tile([65, 2, 512], f32, tag="av",
                                name=f"av{ic}_{p}")
                glist = [("full", jt) for jt in range(4 * ic)]
                glist += [("d0",), ("d1",), ("d23",)]
                sg_w = first_sg if first_sg is not None \
                    else emit_scores(ic, p, glist[0])
                ret = None
                if ic >= 2 and prev_evac is not None:
                    # pair starts in the exp-bound late chunks stall ~0.6us
                    # on the first exps; front-load one extra filler unit
                    fill.credit += 1.0
                for i, g in enumerate(glist):
                    cur = sg_w
                    et = emit_exp(ic, p, g, cur)
                    if i == 0 and prev_evac is not None:
                        # previous pair's deferred avt evacuation, emitted
                        # after this pair's first exp so neither the ACT nor
                        # the DVE queue stalls that exp's consumers; the AV
                        # matmuls below (avt writers) stay correctly ordered
                        # after these reads in program order.
                        prev_evac()
                    if i + 1 < len(glist):
                        sg_w = emit_scores(ic, p, glist[i + 1])
                    elif lookahead is not None:
                        # cross-pair/cross-chunk lookahead: issue the next
                        # pair's first score group now so the exp pipeline
                        # never drains at the pair boundary
                        ic2, p2 = lookahead
                        ret = emit_scores(ic2, p2, first_group(ic2))
                    if i + 2 < len(glist):
                        fill.step()
                    emit_av(ic, p, g, et, avt, first=(i == 0))
                fill.step()
                fill.step()
                return ret, avt

            # ---------------- main schedule -------------------------------
            # startup: chunk-0 qk units, then chunk-1 qk units (their fp8
            # inputs land early) to cover the window while the bigger bf16
            # xt0 + wv still stream in for the v units
            us0 = qkv_units(0)
            us1 = qkv_units(1)
            us2 = qkv_units(2)
            us3 = qkv_units(3)
            us0[0]()
            us0[1]()
            gated_prefetch()
            for u in us0[2:6]:
                u()
            for u in us1[:6]:
                u()
            for u in us0[6:]:
                u()
            fill = Filler()

            # filler placement: remaining qkv during chunks 0-2; the
            # norm/proj units lean late (chunk 3 is exp/ACT-bound, so PE
            # filler there is free, while chunks 1-2 are closer to PE-bound)
            np0 = norm_proj_units(0)
            chunk_fill = {
                0: (us1[6:] + us2[:6], 9),
                1: (us2[6:] + us3[:6], 21),
                2: (us3[6:] + np0, 33),
                3: (norm_proj_units(1) + norm_proj_units(2), 45),
            }

            nxt, evac = None, None
            for ic in range(NIC):
                us, gl = chunk_fill[ic]
                fill.add(us, gl)
                for p in range(NP):
                    if p + 1 < NP:
                        la = (ic, p + 1)
                    elif ic + 1 < NIC:
                        la = (ic + 1, 0)
                    else:
                        la = None
                    nxt, avt = emit_attn_pair(ic, p, fill, prev_evac=evac,
                                              first_sg=nxt, lookahead=la)
                    # 1/S is batched per chunk at the chunk boundary; chunk
                    # 3 splits it: pairs 0-1 normalize during pair 2's
                    # window, pair 2's own chain runs in the tail
                    extra = None
                    if p == NP - 1 and ic < NIC - 1:
                        extra = (lambda ic=ic: emit_lnexp(ic))
                    elif ic == NIC - 1 and p == 1:
                        def extra():
                            # only the ACT chain runs here; the norm units'
                            # bcast matmuls depend on it, so they go to the
                            # FRONT of the filler list instead — emitting
                            # them now would stall the in-order PE queue
                            emit_lnexp(3, rows=slice(0, 33))
                            # insert a few units deep: by the time these
                            # pop, the 1/S chain above has produced rr
                            fill.units[3:3] = [norm_unit(3, pp, h)
                                               for pp in (0, 1)
                                               for h in (0, 1)]
                    evac = make_evac(ic, p, avt, extra)

            # ---------------- tail ----------------------------------------
            # last pair's evacuation + the chunk-3 normalize chain; keepalive
            # matmul bursts bridge the serial ACT/DVE work so the PE clock
            # gate stays at full speed for the final projections.
            def warm_burst(n, nm):
                wt = psgp.tile([128, 2, 512], f32, tag="sg", name=nm)
                for i in range(n):
                    nc.tensor.matmul(wt[:, i % 2, 0:128], warm_src[:],
                                     warm_src[:], start=True, stop=True)

            evac()
            emit_lnexp(3, rows=slice(64, 65))
            warm_burst(56, "wb0")
            fill.flush()
            for h in (0, 1):
                norm_unit(3, 2, h)()
            warm_burst(16, "wb1")
            for tsub in range(4):
                proj_unit(3, tsub, pool=psgp if tsub % 2 else None)()
            # hold the PE clock into the store drain; allocate from the
            # (long-free) AV psum pool — the psg/psc rings are still being
            # evacuated by the tail projections, and a burst allocated there
            # would stall behind those evacuations instead of bridging
            wt = pavp.tile([65, 2, 512], f32, tag="av", name="wbf")
            for i in range(44):
                nc.tensor.matmul(wt[0:65, i % 2, 0:128], warm_src[:, 0:65],
                                 warm_src[:], start=True, stop=True)

    _bacc_mod.get_activation_tables = _pinned_gat
    try:
        nc.compile()
    finally:
        _bacc_mod.get_activation_tables = _orig_gat
    return nc


def make_in_maps(x, w_attn, b_attn, w_proj, b_proj, T=T_FULL):
    import ml_dtypes
    bf = ml_dtypes.bfloat16
    f8e4 = ml_dtypes.float8_e4m3
    x = np.asarray(x, np.float32)
    w_attn = np.asarray(w_attn, np.float32)
    b_attn = np.asarray(b_attn, np.float32)
    w_proj = np.asarray(w_proj, np.float32)
    b_proj = np.asarray(b_proj, np.float32)
    B = x.shape[0]

    ones = np.ones((128, 128), bf)
    # tril mask for the leading 128-column diagonal sub-block of each
    # stripe matmul (two identical copies, one per head)
    mask = np.broadcast_to(
        (np.arange(128)[:, None, None] <= np.arange(128)[None, None, :]),
        (128, 2, 128)).astype(np.float32)

    in_maps = []
    for c in range(N_CORES):
        b, g = (c // 2) % B, c % 2
        q0, k0, v0 = 384 * g, 768 + 384 * g, 1536 + 384 * g
        wqk = np.concatenate(
            [w_attn[:, q0:q0 + 384], w_attn[:, k0:k0 + 384]], axis=1)
        bqk = np.concatenate(
            [b_attn[q0:q0 + 384], b_attn[k0:k0 + 384]])
        # pack to [128 partitions, ...] with k-tile (and chunk) free dims so
        # every DMA line is a 2.3-6KB contiguous run (small lines are slow)
        KT, NIC = 6, T // 512
        xT = x[b].T.reshape(KT, 128, NIC, 512).transpose(1, 2, 0, 3)
        xT = np.ascontiguousarray(xT).reshape(128, -1)
        wqk_p = wqk.reshape(KT, 128, 768).transpose(1, 0, 2)
        wv_p = w_attn[:, v0:v0 + 384].reshape(KT, 128, 384).transpose(1, 0, 2)
        wp_p = (w_proj[384 * g:384 * (g + 1), :]
                .reshape(3, 128, 768).transpose(1, 0, 2))
        in_maps.append({
            "xT": xT.astype(bf),
            "xf8": xT.astype(f8e4),
            "wqk": np.ascontiguousarray(wqk_p).reshape(128, -1).astype(f8e4),
            "wv": np.ascontiguousarray(wv_p).reshape(128, -1).astype(bf),
            "wp": np.ascontiguousarray(wp_p).reshape(128, -1).astype(bf),
            "bqk": np.ascontiguousarray(bqk.reshape(6, 128).T),
            "blob": np.ascontiguousarray(np.concatenate([
                ones,
                mask.reshape(128, 256),
                np.broadcast_to(b_attn[v0:v0 + 384], (128, 384)),
                np.broadcast_to(b_proj if g == 0 else np.zeros_like(b_proj),
                                (128, 768)),
            ], axis=1).astype(np.float32)).astype(bf),
        })
    return in_maps


def kernel(x, w_attn, b_attn, w_proj, b_proj):
    global LAST_RESULT
    if "nc" not in _NC_CACHE:
        _NC_CACHE["nc"] = build_nc(T_FULL)
    nc = _NC_CACHE["nc"]
    in_maps = make_in_maps(x, w_attn, b_attn, w_proj, b_proj)
    res = bass_utils.run_bass_kernel_spmd(
        nc, in_maps, core_ids=list(range(N_CORES)), trace=TRACE)
    LAST_RESULT = res
    B, T, C = np.asarray(x).shape
    y = np.empty((B, T, C), np.float32)
    for b in range(B):
        y[b] = (res.results[2 * b]["y"].astype(np.float32)
                + res.results[2 * b + 1]["y"].astype(np.float32))
    return y

